# revision 3
# baseline (speedup 1.0000x reference)
"""Trainium2 kernel for nn_CorticalColumnLinear.

Computes out[b,s,o] = x[b,s,:] @ (weight*mask)[o,:] with
x [8,4096,1024] f32, weight/mask [1024,1024] f32.

Strategy: pure data-parallel over the batch dim — core i handles x[i]
([4096,1024] @ [1024,1024]^T). The masked weight is replicated.

The mask is 2:4 structured along the INPUT dim: for each group of 4
input columns, 2 are active for ALL output rows.  So (weight*mask)
has only 512 nonzero input columns — the host drops the dead half of
the contraction (and the matching columns of x), halving PE work.

Per-core kernel (v2, trace-tuned):
  - host computes the masked weight, compacts contraction 1024->512,
    pre-transposes x (no PE transposes on device), and casts both
    operands to bf16 (tolerance is 2e-2; bf16 lands ~4e-3).
  - all DMA traffic is packed host-side into partition-major blocks so
    every transfer is 128 contiguous per-partition lines (single
    trigger, line-merged descriptors): x as 4 ramped chunk-blocks
    [128, kt, mc], w as 2 oc-half blocks [128, kt, 512], out as
    bundled m-tile groups into a [128, 32, 1024] DRAM layout the host
    transposes back.
  - device: everything lives in SBUF; the kernel is a pure matmul
    stream — 256 MMs of N=512 bf16 (4-deep k-accumulation), measured
    at the warm 216 ns/MM roofline.  PE floor 55.3 us.
  - PSUM evictions (fp32->bf16) alternate scalar/vector engines;
    output bundles alternate the two HWDGE rings; the last bundles
    shrink (2,1,1) to cut the drain tail.
  - 8 dummy-MM warmup covers the initial DMA fill and flips the PE
    HAM clock-gate to 8/8 right when the real matmuls start.
"""

import numpy as np
import ml_dtypes

import concourse.mybir as mybir
import concourse.tile as tile
from concourse import bacc
from concourse.bass_utils import run_bass_kernel_spmd

F32 = mybir.dt.float32
BF16 = mybir.dt.bfloat16
BF16NP = np.dtype(ml_dtypes.bfloat16)

B, S, D_IN, D_OUT = 8, 4096, 1024, 1024
P = 128
FD = 512   # matmul moving free dim (one PSUM bank of fp32)

_NC_CACHE = {}


def _chunks(s):
    """x DMA chunk sizes along m: small first so MMs start early."""
    if s >= 4096:
        return [512, 512, 1024, s - 2048]
    out, rem, c = [], s, min(256, s)
    while rem:
        c = min(c, rem)
        out.append(c)
        rem -= c
        c *= 2
    return out


def _bundles(mt_n):
    """Output store bundle sizes (in m-tiles); small at the end."""
    bs, rem = [], mt_n
    while rem > 4:
        bs.append(4)
        rem -= 4
    if rem == 4:
        bs += [2, 1, 1]
    elif rem == 3:
        bs += [2, 1]
    else:
        bs += [1] * rem
    return bs


def build_program(s=S, kc=512):
    kt_n = kc // P
    mt_n = s // P
    chunks = _chunks(s)
    bundles = _bundles(mt_n)

    nc = bacc.Bacc("TRN2", target_bir_lowering=False)
    xbs_d = [
        nc.dram_tensor(f"xb{ci}", [P, kt_n, mc], BF16, kind="ExternalInput")
        for ci, mc in enumerate(chunks)
    ]
    wA_d = nc.dram_tensor("wA", [P, kt_n, FD], BF16, kind="ExternalInput")
    wB_d = nc.dram_tensor("wB", [P, kt_n, FD], BF16, kind="ExternalInput")
    out_d = nc.dram_tensor("out", [P, mt_n, D_OUT], BF16, kind="ExternalOutput")

    with tile.TileContext(nc) as tc:
        with (
            tc.tile_pool(name="wpool", bufs=1) as wpool,
            tc.tile_pool(name="xpool", bufs=1) as xpool,
            tc.tile_pool(name="opool", bufs=3) as opool,
            tc.tile_pool(name="warmp", bufs=1) as warmp,
            tc.tile_pool(name="ps", bufs=6, space="PSUM") as ps,
            tc.tile_pool(name="pswarm", bufs=1, space="PSUM") as pswarm,
        ):
            # HAM warmup: junk MMs on a zeroed scratch tile keep the PE
            # busy during the initial DMA fill so the clock gate is at
            # 8/8 (2.4 GHz) when the real matmuls arrive.
            scratch = warmp.tile([P, FD], BF16)
            nc.vector.memset(scratch[:], 0)
            wps = pswarm.tile([P, FD], F32)
            for _ in range(8):
                nc.tensor.matmul(
                    wps[:], scratch[:, 0:P], scratch[:], start=True, stop=True
                )

            # Weights gate the first accumulation chains: sync ring, first.
            wts = []
            for name, wd in (("wa", wA_d), ("wb", wB_d)):
                wt_t = wpool.tile([P, kt_n, FD], BF16, name=name)
                nc.sync.dma_start(wt_t[:], wd[:])
                wts.append(wt_t)

            # x chunk blocks ride the scalar ring.
            xts = []
            for ci, mc in enumerate(chunks):
                xt_t = xpool.tile([P, kt_n, mc], BF16, name=f"xt{ci}")
                nc.scalar.dma_start(xt_t[:], xbs_d[ci][:])
                xts.append(xt_t)

            # m-tile -> (chunk, local tile) map
            locs = []
            for ci, mc in enumerate(chunks):
                locs += [(ci, j) for j in range(mc // P)]

            mt = 0
            for bi, G in enumerate(bundles):
                ob = opool.tile([P, G, D_OUT], BF16, tag="ob")
                for g in range(G):
                    ci, j = locs[mt]
                    for oc in range(2):
                        acc = ps.tile([P, FD], F32, tag="acc")
                        for kt in range(kt_n):
                            nc.tensor.matmul(
                                acc[:],
                                xts[ci][:, kt, j * P:(j + 1) * P],
                                wts[oc][:, kt, :],
                                start=(kt == 0),
                                stop=(kt == kt_n - 1),
                            )
                        # scalar+vector can hit PSUM in parallel on
                        # different banks; split evictions between them.
                        if oc == 0:
                            nc.scalar.copy(ob[:, g, 0:FD], acc[:])
                        else:
                            nc.vector.tensor_copy(out=ob[:, g, FD:D_OUT], in_=acc[:])
                    mt += 1
                eng = nc.sync if bi % 2 == 0 else nc.scalar
                eng.dma_start(out_d[:, mt - G:mt, :], ob[:])

    nc.finalize()
    return nc


def _get_program(s, kc):
    key = (s, kc)
    if key not in _NC_CACHE:
        _NC_CACHE[key] = build_program(s, kc)
    return _NC_CACHE[key]


def _prep(x, weight, mask):
    """Host prep: mask, compact dead input columns, transpose, pack
    partition-major DMA blocks, bf16-cast.  Returns per-core in_maps."""
    x = np.asarray(x, dtype=np.float32)
    weight = np.asarray(weight, dtype=np.float32)
    mask = np.asarray(mask, dtype=np.float32)
    s = x.shape[1]

    w = weight * mask                        # exact elementwise product
    act = np.flatnonzero(mask.any(axis=0))   # live input columns
    kc = len(act)
    kcp = max(P, -(-kc // P) * P)            # pad to multiple of 128
    kt_n = kcp // P

    wtc = np.zeros((kcp, D_OUT), dtype=np.float32)
    wtc[:kc] = w[:, act].T
    wA = np.ascontiguousarray(
        wtc[:, :FD].reshape(kt_n, P, FD).transpose(1, 0, 2).astype(BF16NP))
    wB = np.ascontiguousarray(
        wtc[:, FD:].reshape(kt_n, P, FD).transpose(1, 0, 2).astype(BF16NP))

    chunks = _chunks(s)
    in_maps = []
    for i in range(x.shape[0]):
        xtp = np.zeros((kcp, s), dtype=np.float32)
        xtp[:kc] = x[i].T[act]
        m = {"wA": wA, "wB": wB}
        off = 0
        for ci, mc in enumerate(chunks):
            m[f"xb{ci}"] = np.ascontiguousarray(
                xtp[:, off:off + mc].reshape(kt_n, P, mc)
                .transpose(1, 0, 2).astype(BF16NP))
            off += mc
        in_maps.append(m)
    return in_maps, s, kcp


def run(x, weight, mask, trace=False):
    in_maps, s, kcp = _prep(x, weight, mask)
    nc = _get_program(s, kcp)
    res = run_bass_kernel_spmd(nc, in_maps, list(range(len(in_maps))), trace=trace)
    out = np.stack(
        [
            np.asarray(res.results[i]["out"])
            .transpose(1, 0, 2).reshape(s, D_OUT).astype(np.float32)
            for i in range(len(in_maps))
        ],
        axis=0,
    )
    return out, res


def kernel(x, weight, mask):
    out, _ = run(x, weight, mask)
    return out
